# revision 28
# baseline (speedup 1.0000x reference)
"""Trainium2 Bass kernel for nn_Attention_25915832664752.

Reference computation (per reference.py):
    For b in {Q,K,V}:  q0 = relu(IN(conv1d(Z, W[b,0])));  q1 = relu(IN(conv1d(Z, W[b,1]) @ L))
                       X_b = q0 + q1                                  [2048, 48]
    A  = exp(Q @ K^T)                                                 [2048, 2048]
    P  = A / rowsum(A);  Aa = (P + P^T)/2;  out = Aa @ V              [2048, 48]

Strategy (8 NeuronCores, tensor-parallel over nhid):
    Core c owns output channels [c*256, (c+1)*256).  W is quantized on the
    host to fp8 e4m3 (x2^20 scale; instance norm makes the scale exact by
    using EPS*scale^2) with a data-aware error-feedback pass: weights are
    rounded sequentially per output channel, each to the fp8 neighbor that
    cancels the accumulated conv-output error against the actual stationary
    operand values.  The stationaries (padded-Z windows and ZcolL = Zcol@L)
    are stored as TWO-LEVEL fp8 (value = hi + lo, lo = fp8 of the residual)
    so the conv runs entirely in fp8 with DoubleRow perf mode (2 k-tiles
    per instruction, 0.5 cycles/row): bf16-level accuracy at fp8 bytes and
    2x PE rate.  W is read from HBM exactly once (28.3 MB/core).

    Schedule: three k-streaming sweeps K -> Q -> V.  K's all-gather hides
    under the Q sweep; kT / exp(Q K^T) / rowsums / exp(K Q^T) hide under
    the V sweep.  Tail: V epilogue, U = A^T (0.5 rinv V_loc) -> bf16
    ReduceScatter concurrent with the bf16 V all-gather ->
    0.5 rinv*(At^T-contract V_full) + U_scattered, store.
"""

import os
import sys

import numpy as np

sys.path.insert(0, "/opt/trn_rl_repo")

import orjson

import concourse.bass as bass
import concourse.mybir as mybir
from concourse import masks, tile
from concourse.bass_utils import run_bass_kernel_spmd

# ---------------------------------------------------------------- waitfix ---
# This neuronxcc build allows only ONE sync wait per instruction;
# TileContext emits instructions with several.  Rewrite the serialized BIR:
# hoist extra waits onto standalone NoOps inserted just before the
# instruction on the same engine (cumulative thresholds -> semantics kept).

_DMA_OPCODES = {
    "DMACopy", "DMATranspose", "TensorLoad", "TensorSave",
    "TriggeredCopy", "CollectiveCompute",
}
_wfix_counter = [0]


def _fix_block(instructions):
    out = []
    for ins in instructions:
        si = ins.get("sync_info")
        if not si:
            out.append(ins)
            continue
        waits = si.get("on_wait") or []
        updates = si.get("on_update") or []
        if len(waits) > 1:
            for w in waits[1:]:
                _wfix_counter[0] += 1
                out.append({
                    "engine": ins["engine"], "ins": [],
                    "name": f"WFIX-{_wfix_counter[0]}", "opcode": "NoOp",
                    "outs": [],
                    "sync_info": {"on_update": [], "on_wait": [w]},
                })
            si["on_wait"] = waits[:1]
        deferred = []
        if len(updates) > 1:
            assert ins.get("opcode", "") not in _DMA_OPCODES, (
                f"multi-update on DMA opcode: {ins['name']}"
            )
            si["on_update"] = updates[:1]
            for u in updates[1:]:
                _wfix_counter[0] += 1
                deferred.append({
                    "engine": ins["engine"], "ins": [],
                    "name": f"WFIX-{_wfix_counter[0]}", "opcode": "NoOp",
                    "outs": [],
                    "sync_info": {"on_update": [u], "on_wait": []},
                })
        out.append(ins)
        out.extend(deferred)
    return out


def _fix_bir_json_bytes(data: bytes) -> bytes:
    d = orjson.loads(data)
    for func in d.get("functions", []):
        for bb in func.get("blocks", []):
            bb["instructions"] = _fix_block(bb["instructions"])
    return orjson.dumps(d)


if not getattr(bass.Bass, "_waitfix_installed", False):
    _orig_to_json_bytes = bass.Bass.to_json_bytes

    def _patched_to_json_bytes(self) -> bytes:
        return _fix_bir_json_bytes(_orig_to_json_bytes(self))

    bass.Bass.to_json_bytes = _patched_to_json_bytes
    bass.Bass._waitfix_installed = True

# Synthesize the missing ``antenv.axon_hooks`` module so that
# ``run_bass_kernel_spmd(trace=True)`` can drive NTFF profiling through the
# axon PJRT plugin (the boot-time registration degrades silently when the
# module is absent).  Harmless when tracing is never requested.
try:
    import types

    import antenv

    if not hasattr(antenv, "axon_hooks"):
        _hooks_mod = types.ModuleType("antenv.axon_hooks")
        _ntff_hook = [None]
        _hooks_mod.set_axon_ntff_profile_hook = lambda h: _ntff_hook.__setitem__(0, h)
        _hooks_mod.get_axon_ntff_profile_hook = lambda: _ntff_hook[0]
        sys.modules["antenv.axon_hooks"] = _hooks_mod
        antenv.axon_hooks = _hooks_mod
        from trn_agent_boot.trn_boot import _ntff_profile_via_ctypes

        _hooks_mod.set_axon_ntff_profile_hook(
            _ntff_profile_via_ctypes("/opt/axon/libaxon_pjrt.so"))

    import concourse.bass_utils as _bu

    _bu.upload_artifacts = lambda tmpdir: tmpdir  # no fish share in container
except Exception:  # pragma: no cover - profiling is best-effort
    pass



# ------------------------------------------------------------- constants ---

NHID = 2048
NOPEN = 2048
N = 48          # spatial length
KD = 9          # conv kernel width
PAD = 4
NP = N + 2 * PAD            # 56 padded spatial
EPS = 1e-5
CORES = 8
OLOC = NHID // CORES        # 256 output channels per core
KTOT = KD * NOPEN           # 18432 contraction length
NKT = KTOT // 128           # 144 k-tiles
ISUB = NOPEN // 128         # 16 i-subtiles
SCALE = float(2.0 ** 20)    # fp8 weight scale; exact via EPS_S below
EPS_S = EPS * SCALE * SCALE
CK = 16                     # k-tiles per W DMA chunk (1 MB fp8)
NCH = NKT // CK             # 9 chunks per sweep
SCOLS = 512                 # cols per k-tile per sweep: [g_r0 | g_r1]
F32 = mybir.dt.float32
F32R = mybir.dt.float32r
BF16 = mybir.dt.bfloat16
FP8 = mybir.dt.float8e4
DR = mybir.MatmulPerfMode.DoubleRow


def _build_nc():
    nc = bass.Bass()

    wt_d = {b: nc.declare_dram_parameter(
        f"wt{b}", [NCH, 128, CK * SCOLS], FP8, isOutput=False)
        for b in range(3)}
    # pair-packed conv stationaries: slice for k-tile pair pi = t*8+sp is
    # [win(t, 2sp) | win(t, 2sp+1)], 96 wide (DoubleRow lhsT must be packed)
    zh_d = nc.declare_dram_parameter("zh", [128, NKT * N], FP8,
                                     isOutput=False)
    zl_d = nc.declare_dram_parameter("zl", [128, NKT * N], FP8,
                                     isOutput=False)
    zpt_d = nc.declare_dram_parameter("zpt", [NP, NOPEN], BF16, isOutput=False)
    lpd_d = nc.declare_dram_parameter("lpd", [NP, KD * N], BF16, isOutput=False)
    out_d = nc.declare_dram_parameter("out", [OLOC, N], F32, isOutput=True)

    with tile.TileContext(nc) as tc:
        with (
            tc.tile_pool(name="pers", bufs=1) as pers,
            tc.tile_pool(name="wpk", bufs=5) as wpk,
            tc.tile_pool(name="wpq", bufs=5) as wpq,
            tc.tile_pool(name="wpv", bufs=5) as wpv,
            tc.tile_pool(name="stats", bufs=1) as stats,
            tc.tile_pool(name="pacc", bufs=2, space="PSUM") as pacc,
            tc.tile_pool(name="ptrans", bufs=2, space="PSUM") as ptrans,
            tc.tile_pool(name="dram", bufs=1, space="DRAM") as dram,
        ):
            # ---------------- prologue ------------------------------------
            ident = pers.tile([128, 128], F32, tag="ident")
            masks.make_identity(nc, ident[:])
            identb = pers.tile([128, 128], BF16, tag="identb")
            nc.vector.tensor_copy(identb[:], ident[:])

            # rendezvous: absorb cross-core launch skew under the K sweep
            rg = [list(range(CORES))]
            rdv_in = dram.tile([2, 4], F32, tag="rdv_in")
            rdv_out = dram.tile([16, 4], F32, tag="rdv_out", addr_space="Shared")
            nc.gpsimd.collective_compute(
                "AllGather", mybir.AluOpType.bypass,
                replica_groups=rg, ins=[rdv_in.opt()], outs=[rdv_out.opt()])

            # two-level fp8 padded-Z windows (host-prepadded, pair-packed).
            # Issued on the sync queue AHEAD of the W chunk stream: on a
            # side queue they starve behind 6 MB of W prefetch and stall
            # the first conv matmul for ~18 us.
            zpt = pers.tile([128, NOPEN], BF16, tag="zpt")
            nc.sync.dma_start(out=zpt[0:NP, :], in_=zpt_d[:])
            lpd = pers.tile([128, KD * N], BF16, tag="lpd")
            nc.sync.dma_start(out=lpd[0:NP, :], in_=lpd_d[:])
            zh = pers.tile([128, NKT * N], FP8, tag="zh")
            nc.sync.dma_start(out=zh[:], in_=zh_d[:])
            zl = pers.tile([128, NKT * N], FP8, tag="zl")
            nc.sync.dma_start(out=zl[:], in_=zl_d[:])

            # ZcolL[(t,i), n'] = sum_n Zpad[i, n+t] L[n, n'], two-level fp8,
            # written pair-packed: [p][t, sp, parity, n]
            zch = pers.tile([128, NKT * N], FP8, tag="zch")
            zcl = pers.tile([128, NKT * N], FP8, tag="zcl")
            zch_w = zch[:].rearrange("p (t q r x) -> p t q r x",
                                     q=8, r=2, x=N)
            zcl_w = zcl[:].rearrange("p (t q r x) -> p t q r x",
                                     q=8, r=2, x=N)
            scr2 = [pers.tile([128, KD * N], F32, tag=f"zscr{i}",
                              name=f"zscr{i}") for i in range(2)]
            for s in range(ISUB):
                ps = ptrans.tile([128, KD * N], F32, tag="pzcl",
                                 name=f"pzcl{s}", bufs=2)
                nc.tensor.matmul(
                    ps[:, 0:KD * N],
                    zpt[0:NP, s * 128:(s + 1) * 128],
                    lpd[0:NP, :],
                    start=True, stop=True,
                )
                sp, par = s // 2, s % 2
                ps_v = ps[:, 0:KD * N].rearrange("p (t x) -> p t x", x=N)
                hi_ap = zch_w[:, :, sp, par, :]
                nc.scalar.copy(hi_ap, ps_v)
                scr_v = scr2[s % 2][:].rearrange("p (t x) -> p t x", x=N)
                nc.vector.tensor_tensor(scr_v, ps_v, hi_ap,
                                        op=mybir.AluOpType.subtract)
                nc.scalar.copy(zcl_w[:, :, sp, par, :], scr_v)

            zh_v = zh[:].rearrange("p (q x) -> p q x", x=N)    # [128,288,48]
            zl_v = zl[:].rearrange("p (q x) -> p q x", x=N)
            zch_v = zch[:].rearrange("p (q x) -> p q x", x=N)
            zcl_v = zcl[:].rearrange("p (q x) -> p q x", x=N)

            # ---------------- conv sweeps ---------------------------------
            relu_sc = pers.tile([128, 12 * N], F32, tag="relu_sc")
            yt_sb = pers.tile([128, 2 * 256], F32, tag="yt_sb")

            def conv_sweep(b, wpool_, accs, c0, c1, two_level):
                """One tensor's k-stream.  Per k-tile pair: one DoubleRow
                matmul per (branch, level); V uses the hi level only."""
                levels0 = (zh_v, zl_v) if two_level else (zh_v,)
                levels1 = (zch_v, zcl_v) if two_level else (zch_v,)
                nlev = len(levels0)
                for gch in range(c0, c1):
                    wt = wpool_.tile([128, CK * SCOLS], FP8, tag=f"w{b}",
                                     name=f"w{b}c{gch}")
                    nc.sync.dma_start(out=wt[:], in_=wt_d[b][gch])
                    wvv = wt[:].rearrange("p (j x) -> p j x", x=SCOLS)
                    for j in range(0, CK, 2):
                        kt = gch * CK + j
                        t, s = kt // ISUB, kt % ISUB
                        pi = t * 8 + s // 2
                        st = (kt == 0)
                        sp = (kt == NKT - 2)
                        for lv in range(nlev):
                            nc.tensor.matmul(
                                accs[0][0:N, :],
                                levels0[lv][:, 2 * pi:2 * pi + 2, :],
                                wvv[:, j:j + 2, 0:256],
                                start=(st and lv == 0),
                                stop=(sp and lv == nlev - 1),
                                perf_mode=DR)
                            nc.tensor.matmul(
                                accs[1][0:N, :],
                                levels1[lv][:, 2 * pi:2 * pi + 2, :],
                                wvv[:, j:j + 2, 256:512],
                                start=(st and lv == 0),
                                stop=(sp and lv == nlev - 1),
                                perf_mode=DR)

            def sweep_epilogue(entries, label):
                """entries: list of (g, acc_ap[48, 256]).  Transpose each
                half to [128, 48], then batched instance-norm stats + fused
                relu.  relu_sc slot = g*2 + h."""
                nslot = 2 * len(entries)
                xc = stats.tile([128, nslot * N], F32, tag=f"xc{label}",
                                name=f"xc{label}")
                slots = []
                for idx, (g, acc_ap) in enumerate(entries):
                    nc.scalar.copy(
                        yt_sb[0:N, idx * OLOC:(idx + 1) * OLOC], acc_ap)
                    for h in range(2):
                        ot = g * 2 + h
                        slot = idx * 2 + h
                        ps2 = ptrans.tile([128, 128], F32, tag="ptrans",
                                          name=f"tp{label}{ot}")
                        nc.tensor.transpose(
                            ps2[:, 0:N],
                            yt_sb[0:N, idx * OLOC + h * 128:
                                  idx * OLOC + (h + 1) * 128],
                            ident[0:N, 0:N])
                        nc.scalar.copy(xc[:, slot * N:(slot + 1) * N],
                                       ps2[:, 0:N])
                        slots.append((ot, slot))
                sm = stats.tile([128, nslot], F32, tag=f"sm{label}",
                                name=f"sm{label}")
                sq = stats.tile([128, nslot], F32, tag=f"sq{label}",
                                name=f"sq{label}")
                scr = stats.tile([128, nslot * N], F32, tag=f"scr{label}",
                                 name=f"scr{label}")
                for ot, slot in slots:
                    nc.vector.reduce_sum(
                        sm[:, slot:slot + 1], xc[:, slot * N:(slot + 1) * N],
                        axis=mybir.AxisListType.X)
                nc.vector.tensor_tensor(scr[:], xc[:], xc[:],
                                        op=mybir.AluOpType.mult)
                for ot, slot in slots:
                    nc.vector.reduce_sum(
                        sq[:, slot:slot + 1], scr[:, slot * N:(slot + 1) * N],
                        axis=mybir.AxisListType.X)
                mean = stats.tile([128, nslot], F32, tag=f"mean{label}",
                                  name=f"mean{label}")
                var = stats.tile([128, nslot], F32, tag=f"var{label}",
                                 name=f"var{label}")
                std = stats.tile([128, nslot], F32, tag=f"std{label}",
                                 name=f"std{label}")
                rsv = stats.tile([128, nslot], F32, tag=f"rsv{label}",
                                 name=f"rsv{label}")
                nb = stats.tile([128, nslot], F32, tag=f"nb{label}",
                                name=f"nb{label}")
                nc.vector.tensor_scalar_mul(mean[:], sm[:], 1.0 / N)
                nc.vector.tensor_scalar_mul(sq[:], sq[:], 1.0 / N)
                nc.vector.tensor_tensor(var[:], mean[:], mean[:],
                                        op=mybir.AluOpType.mult)
                nc.vector.tensor_tensor(var[:], sq[:], var[:],
                                        op=mybir.AluOpType.subtract)
                nc.vector.tensor_scalar_add(var[:], var[:], EPS_S)
                nc.scalar.sqrt(std[:], var[:])
                nc.vector.reciprocal(rsv[:], std[:])
                nc.vector.tensor_tensor(nb[:], mean[:], rsv[:],
                                        op=mybir.AluOpType.mult)
                nc.vector.tensor_scalar_mul(nb[:], nb[:], -1.0)
                for ot, slot in slots:
                    nc.scalar.activation(
                        relu_sc[:, ot * N:(ot + 1) * N],
                        xc[:, slot * N:(slot + 1) * N],
                        mybir.ActivationFunctionType.Relu,
                        bias=nb[:, slot:slot + 1], scale=rsv[:, slot:slot + 1])

            def branch_add(b, dst):
                for h in range(2):
                    ot0 = (2 * b) * 2 + h        # r = 0
                    ot1 = (2 * b + 1) * 2 + h    # r = 1
                    nc.vector.tensor_tensor(
                        dst[:, h * N:(h + 1) * N],
                        relu_sc[:, ot0 * N:(ot0 + 1) * N],
                        relu_sc[:, ot1 * N:(ot1 + 1) * N],
                        op=mybir.AluOpType.add)

            # ---- sweep 1: K (b=1); its gather hides under Q + V ----------
            accK = [pacc.tile([128, 256], F32, tag="acc", name=f"accK{i}")
                    for i in range(2)]
            conv_sweep(1, wpk, accK, 0, NCH, True)
            sweep_epilogue([(2, accK[0][0:N, :]), (3, accK[1][0:N, :])], "K")
            k_sb = pers.tile([128, 2 * N], BF16, tag="k_sb")
            branch_add(1, k_sb)

            kb = dram.tile([OLOC, N], BF16, tag="kb")
            kg = dram.tile([NHID, N], BF16, tag="kg", addr_space="Shared")
            nc.scalar.dma_start(
                out=kb[:].rearrange("(a p) n -> p a n", p=128),
                in_=k_sb[:].rearrange("p (a n) -> p a n", n=N))
            nc.gpsimd.collective_compute(
                "AllGather", mybir.AluOpType.bypass,
                replica_groups=rg, ins=[kb.opt()], outs=[kg.opt()])

            # ---- sweep 2: Q (b=0) ----------------------------------------
            accQ = [pacc.tile([128, 256], F32, tag="acc", name=f"accQ{i}")
                    for i in range(2)]
            conv_sweep(0, wpq, accQ, 0, NCH, True)
            sweep_epilogue([(0, accQ[0][0:N, :]), (1, accQ[1][0:N, :])], "Q")
            q_sb = pers.tile([128, 2 * N], BF16, tag="q_sb")
            branch_add(0, q_sb)

            qT = pers.tile([128, 2 * 128], BF16, tag="qT")
            for h in range(2):
                ps = ptrans.tile([128, 128], BF16, tag="ptrans", name=f"qT{h}")
                nc.tensor.transpose(
                    ps[0:N, :], q_sb[:, h * N:(h + 1) * N], identb[:])
                nc.scalar.copy(qT[0:N, h * 128:(h + 1) * 128], ps[0:N, :])

            kfull = pers.tile([128, 16 * N], BF16, tag="kfull")
            nc.gpsimd.dma_start(
                out=kfull[:].rearrange("p (a n) -> p a n", n=N),
                in_=kg[:].rearrange("(a p) n -> p a n", p=128))

            # ---- sweep 3: V (b=2, single-level) with attention prework ---
            accV = [pacc.tile([128, 256], F32, tag="acc", name=f"accV{i}")
                    for i in range(2)]
            conv_sweep(2, wpv, accV, 0, 3, False)

            # kT transposes: K gather is done by now
            kT = pers.tile([128, NHID], BF16, tag="kT")
            for jt in range(16):
                ps = ptrans.tile([128, 128], BF16, tag="ptrans",
                                 name=f"kT{jt}")
                nc.tensor.transpose(
                    ps[0:N, :], kfull[:, jt * N:(jt + 1) * N], identb[:])
                nc.scalar.copy(kT[0:N, jt * 128:(jt + 1) * 128], ps[0:N, :])

            conv_sweep(2, wpv, accV, 3, 5, False)

            # A rows: a_sb[m] = exp(Q_m K_full^T) (bf16) + rowsums
            a_sb = [pers.tile([128, NHID], BF16, tag=f"a{m}", name=f"a{m}")
                    for m in range(2)]
            rinvh = []
            for m in range(2):
                rspart = stats.tile([128, 4], F32, tag=f"rsp{m}",
                                    name=f"rsp{m}")
                for jc in range(4):
                    ps = ptrans.tile([128, 512], F32, tag="ptrans",
                                     name=f"am{m}c{jc}")
                    nc.tensor.matmul(
                        ps[:, 0:512],
                        qT[0:N, m * 128:(m + 1) * 128],
                        kT[0:N, jc * 512:(jc + 1) * 512],
                        start=True, stop=True)
                    nc.scalar.activation(
                        a_sb[m][:, jc * 512:(jc + 1) * 512], ps[:, 0:512],
                        mybir.ActivationFunctionType.Exp,
                        accum_out=rspart[:, jc:jc + 1])
                rowsum = stats.tile([128, 1], F32, tag=f"rowsum{m}",
                                    name=f"rowsum{m}")
                nc.vector.reduce_sum(rowsum[:], rspart[:],
                                     axis=mybir.AxisListType.X)
                rinv = stats.tile([128, 1], F32, tag=f"rinv{m}",
                                  name=f"rinv{m}")
                nc.vector.reciprocal(rinv[:], rowsum[:])
                rh = stats.tile([128, 1], F32, tag=f"rinvh{m}",
                                name=f"rinvh{m}")
                nc.vector.tensor_scalar_mul(rh[:], rinv[:], 0.5)
                rinvh.append(rh)

            conv_sweep(2, wpv, accV, 5, 7, False)

            # At = exp(K_full Q_loc^T) [2048, 256] (bf16)
            at_sb = pers.tile([128, 16 * 256], BF16, tag="at")
            for jt in range(16):
                ps = ptrans.tile([128, 256], F32, tag="ptrans",
                                 name=f"at{jt}")
                nc.tensor.matmul(
                    ps[:, 0:256],
                    kT[0:N, jt * 128:(jt + 1) * 128],
                    qT[0:N, 0:256],
                    start=True, stop=True)
                nc.scalar.activation(
                    at_sb[:, jt * 256:(jt + 1) * 256], ps[:, 0:256],
                    mybir.ActivationFunctionType.Exp)

            conv_sweep(2, wpv, accV, 7, NCH, False)

            # ---- V epilogue + tail ---------------------------------------
            sweep_epilogue([(4, accV[0][0:N, :]), (5, accV[1][0:N, :])], "V")
            vloc = pers.tile([128, 2 * N], BF16, tag="vloc")
            branch_add(2, vloc)

            # V all-gather (bf16), concurrent with U/ReduceScatter
            vb = dram.tile([OLOC, N], BF16, tag="vb")
            vg = dram.tile([NHID, N], BF16, tag="vg", addr_space="Shared")
            nc.scalar.dma_start(
                out=vb[:].rearrange("(a p) n -> p a n", p=128),
                in_=vloc[:].rearrange("p (a n) -> p a n", n=N))
            nc.gpsimd.collective_compute(
                "AllGather", mybir.AluOpType.bypass,
                replica_groups=rg, ins=[vb.opt()], outs=[vg.opt()])

            # U = A_loc^T (0.5 rinv V_loc);  ReduceScatter in bf16
            vr = pers.tile([128, 2 * N], BF16, tag="vr")
            for m in range(2):
                nc.vector.tensor_scalar_mul(
                    vr[:, m * N:(m + 1) * N], vloc[:, m * N:(m + 1) * N],
                    rinvh[m][:])
            u_sb = pers.tile([128, 16 * N], BF16, tag="u")
            for jt in range(16):
                ps = ptrans.tile([128, 128], F32, tag="ptrans",
                                 name=f"u{jt}")
                for m in range(2):
                    nc.tensor.matmul(
                        ps[:, 0:N],
                        a_sb[m][:, jt * 128:(jt + 1) * 128],
                        vr[:, m * N:(m + 1) * N],
                        start=(m == 0), stop=(m == 1))
                u_copy = (nc.vector.tensor_copy if jt % 2 == 0
                          else nc.scalar.copy)
                u_copy(u_sb[:, jt * N:(jt + 1) * N], ps[:, 0:N])

            ub = dram.tile([NHID, N], BF16, tag="ub")
            rsb = dram.tile([OLOC, N], BF16, tag="rsb")
            nc.scalar.dma_start(
                out=ub[:].rearrange("(a p) n -> p a n", p=128),
                in_=u_sb[:].rearrange("p (a n) -> p a n", n=N))
            nc.gpsimd.collective_compute(
                "ReduceScatter", mybir.AluOpType.add,
                replica_groups=rg, ins=[ub.opt()], outs=[rsb.opt()])

            vfull = pers.tile([128, 16 * N], BF16, tag="vfull")
            nc.gpsimd.dma_start(
                out=vfull[:].rearrange("p (a n) -> p a n", n=N),
                in_=vg[:].rearrange("(a p) n -> p a n", p=128))

            rs_bf = pers.tile([128, 2 * N], BF16, tag="rs_bf")
            nc.gpsimd.dma_start(
                out=rs_bf[:].rearrange("p (a n) -> p a n", n=N),
                in_=rsb[:].rearrange("(a p) n -> p a n", p=128))
            rs_sb = pers.tile([128, 2 * N], F32, tag="rs_sb")
            nc.vector.tensor_copy(rs_sb[:], rs_bf[:])

            # out = 0.5 rinv * (At^T-contract V_full) + rs
            fin = pers.tile([128, 2 * N], F32, tag="fin")
            for m in range(2):
                ps = ptrans.tile([128, 128], F32, tag="ptrans",
                                 name=f"fin{m}")
                for jt in range(16):
                    nc.tensor.matmul(
                        ps[:, 0:N],
                        at_sb[:, jt * 256 + m * 128: jt * 256 + (m + 1) * 128],
                        vfull[:, jt * N:(jt + 1) * N],
                        start=(jt == 0), stop=(jt == 15))
                nc.vector.scalar_tensor_tensor(
                    out=fin[:, m * N:(m + 1) * N],
                    in0=ps[:, 0:N],
                    scalar=rinvh[m][:],
                    in1=rs_sb[:, m * N:(m + 1) * N],
                    op0=mybir.AluOpType.mult,
                    op1=mybir.AluOpType.add)

            nc.scalar.dma_start(
                out=out_d[:].rearrange("(a p) n -> p a n", p=128),
                in_=fin[:].rearrange("p (a n) -> p a n", n=N))

    return nc


_NC_CACHE = None


def _get_nc():
    global _NC_CACHE
    if _NC_CACHE is None:
        _NC_CACHE = _build_nc()
    return _NC_CACHE


# ------------------------------------------------------- host-side prep ---

def _fp8_cast(x):
    import ml_dtypes

    return np.asarray(x, dtype=ml_dtypes.float8_e4m3)


def _fp8_lo_hi(x):
    """fp8(e4m3) grid values bracketing x (f32 in, f32 out, exact grid)."""
    import ml_dtypes

    dt8 = ml_dtypes.float8_e4m3
    q8 = np.asarray(x, dtype=dt8)
    q = q8.astype(np.float32)
    b = q8.view(np.uint8)
    neg = (b & 0x80) != 0
    up_b = np.where(~neg, b + 1,
                    np.where(b == 0x80, np.uint8(1), b - 1)).astype(np.uint8)
    dn_b = np.where(neg, b + 1,
                    np.where(b == 0, np.uint8(0x81), b - 1)).astype(np.uint8)
    up = up_b.view(dt8).astype(np.float32)
    dn = dn_b.view(dt8).astype(np.float32)
    lo = np.where(q <= x, q, dn)
    hi = np.where(q >= x, q, up)
    return lo, hi


def _make_feedback_numba():
    import numba

    @numba.njit(fastmath=True, boundscheck=False, cache=False)
    def _feedback(Ws, lo, hi, stat, out):
        K, C = Ws.shape
        Nn = stat.shape[1]
        E = np.zeros((C, Nn), np.float32)
        for k in range(K):
            zk = stat[k]
            zz = np.float32(0.0)
            for n in range(Nn):
                zz += zk[n] * zk[n]
            for c in range(C):
                w = Ws[k, c]
                dlo = lo[k, c] - w
                dhi = hi[k, c] - w
                ez = np.float32(0.0)
                Ec = E[c]
                for n in range(Nn):
                    ez += Ec[n] * zk[n]
                if 2 * dlo * ez + dlo * dlo * zz <= 2 * dhi * ez + dhi * dhi * zz:
                    d = dlo
                    out[k, c] = lo[k, c]
                else:
                    d = dhi
                    out[k, c] = hi[k, c]
                for n in range(Nn):
                    Ec[n] += d * zk[n]
        return out

    return _feedback


def _feedback_numpy(Ws, lo, hi, stat, out, nseg=9):
    """Vectorized fallback: segment the k axis, run feedback per segment in
    parallel (slightly worse residual than fully sequential)."""
    K, C = Ws.shape
    Nn = stat.shape[1]
    T = K // nseg
    Wseg = Ws.reshape(nseg, T, C)
    Sseg = stat.reshape(nseg, T, Nn)
    Lseg = lo.reshape(nseg, T, C)
    Hseg = hi.reshape(nseg, T, C)
    Oseg = out.reshape(nseg, T, C)
    E = np.zeros((nseg, C, Nn), np.float32)
    for k in range(T):
        zk = Sseg[:, k]
        dlo = Lseg[:, k] - Wseg[:, k]
        dhi = Hseg[:, k] - Wseg[:, k]
        ez = np.einsum('scn,sn->sc', E, zk)
        zz = (zk * zk).sum(-1)[:, None]
        pick_lo = (2 * dlo * ez + dlo * dlo * zz) <= (2 * dhi * ez + dhi * dhi * zz)
        d = np.where(pick_lo, dlo, dhi)
        Oseg[:, k] = np.where(pick_lo, Lseg[:, k], Hseg[:, k])
        E += d[:, :, None] * zk[:, None, :]
    return out


def _prep_inputs(Z: np.ndarray, L: np.ndarray, W: np.ndarray):
    """Host-side layout + fp8 quantization with conv-output error feedback."""
    import ml_dtypes

    bf = ml_dtypes.bfloat16

    # padded Z, two-level fp8, pair-packed chip layout
    # zh[p, ((t*8+sp)*2+par)*48 + n] = hi(Zpad)[(2sp+par)*128 + p, t + n]
    Zpad = np.zeros((NOPEN, NP), np.float32)
    Zpad[:, PAD:PAD + N] = Z
    z_hi8 = _fp8_cast(Zpad)
    z_hi = z_hi8.astype(np.float32)
    z_lo8 = _fp8_cast(Zpad - z_hi)
    z2 = z_hi + z_lo8.astype(np.float32)        # stationary value on chip

    def pairpack(x8):
        zw = np.empty((128, KD, 8, 2, N), dtype=x8.dtype)
        for t in range(KD):
            for s in range(ISUB):
                zw[:, t, s // 2, s % 2, :] = x8[s * 128:(s + 1) * 128, t:t + N]
        return np.ascontiguousarray(zw.reshape(128, NKT * N))

    zh = pairpack(z_hi8)
    zl = pairpack(z_lo8)

    Zpad_bf = Zpad.astype(bf)
    zpt = np.ascontiguousarray(Zpad_bf.T)                  # [56, 2048]
    lpd = np.zeros((NP, KD * N), np.float32)
    for t in range(KD):
        lpd[t:t + N, t * N:(t + 1) * N] = L
    lpd_bf = lpd.astype(bf)

    # stationary rows as used on chip, k = t*NOPEN + i.  K/Q convs see the
    # two-level value; the V conv uses the hi level only.
    stat0_2 = np.empty((KTOT, N), np.float32)
    stat0_1 = np.empty((KTOT, N), np.float32)
    for t in range(KD):
        stat0_2[t * NOPEN:(t + 1) * NOPEN] = z2[:, t:t + N]
        stat0_1[t * NOPEN:(t + 1) * NOPEN] = z_hi[:, t:t + N]
    # ZcolL replicated: psum f32 of bf16 matmul -> two-level fp8
    zc = (zpt.astype(np.float32).T @ lpd_bf.astype(np.float32))  # [2048, 432]
    zc_hi8 = _fp8_cast(zc)
    zc_hi = zc_hi8.astype(np.float32)
    zc_lo8 = _fp8_cast(zc - zc_hi)
    zc2 = (zc_hi + zc_lo8.astype(np.float32)).reshape(NOPEN, KD, N)
    stat1_2 = np.ascontiguousarray(zc2.transpose(1, 0, 2).reshape(KTOT, N))
    stat1_1 = np.ascontiguousarray(
        zc_hi.reshape(NOPEN, KD, N).transpose(1, 0, 2).reshape(KTOT, N))

    # W -> [k, g, ch] scaled, feedback-quantize per group-set against the
    # stationary values that conv actually uses
    Wt = np.ascontiguousarray(
        W.transpose(4, 3, 0, 1, 2).reshape(KTOT, 6, NHID).astype(np.float32))
    Wt *= SCALE

    try:
        fb = _make_feedback_numba()
        use_numba = True
    except Exception:
        fb = None
        use_numba = False

    Wq = np.empty((KTOT, 6, NHID), np.float32)
    for gsel, stat in (([0, 2], stat0_2), ([1, 3], stat1_2),
                       ([4], stat0_1), ([5], stat1_1)):
        Ws = np.ascontiguousarray(
            Wt[:, gsel, :].reshape(KTOT, len(gsel) * NHID))
        lo, hi = _fp8_lo_hi(Ws)
        outq = np.empty_like(Ws)
        if use_numba:
            fb(Ws, lo, hi, stat, outq)
        else:
            _feedback_numpy(Ws, lo, hi, stat, outq)
        Wq[:, gsel, :] = outq.reshape(KTOT, len(gsel), NHID)
        del Ws, lo, hi, outq
    del Wt

    q8 = _fp8_cast(Wq)           # exact: values are on the fp8 grid
    del Wq

    shards = []
    for c in range(CORES):
        wc = q8[:, :, c * OLOC:(c + 1) * OLOC]       # [k, 6, 256]
        slabs = []
        for b in range(3):
            wb = np.ascontiguousarray(
                wc[:, [2 * b, 2 * b + 1], :]).reshape(NKT, 128, SCOLS)
            wb = np.ascontiguousarray(
                wb.reshape(NCH, CK, 128, SCOLS).transpose(0, 2, 1, 3)
                .reshape(NCH, 128, CK * SCOLS))
            slabs.append(wb)
        shards.append(slabs)

    return (shards, np.ascontiguousarray(zh), np.ascontiguousarray(zl),
            np.ascontiguousarray(zpt), np.ascontiguousarray(lpd_bf))


def kernel(Z: np.ndarray, L: np.ndarray, W: np.ndarray) -> np.ndarray:
    nc = _get_nc()
    shards, zh, zl, zpt, lpd = _prep_inputs(
        np.asarray(Z, dtype=np.float32), np.asarray(L, dtype=np.float32),
        np.asarray(W, dtype=np.float32))
    in_maps = [{"wt0": shards[c][0], "wt1": shards[c][1],
                "wt2": shards[c][2],
                "zh": zh, "zl": zl, "zpt": zpt, "lpd": lpd}
               for c in range(CORES)]
    trace = bool(int(os.environ.get("KERNEL_TRACE", "0")))
    kw = {}
    if trace and int(os.environ.get("KERNEL_TRACE_ALL", "0")):
        kw["trace_cores"] = list(range(CORES))
    res = run_bass_kernel_spmd(nc, in_maps, list(range(CORES)), trace=trace,
                               **kw)
    kernel.last_result = res
    out = np.concatenate([res.results[c]["out"] for c in range(CORES)], axis=0)
    return out


# revision 29
# speedup vs baseline: 1.0146x; 1.0146x over previous
"""Trainium2 Bass kernel for nn_Attention_25915832664752.

Reference computation (per reference.py):
    For b in {Q,K,V}:  q0 = relu(IN(conv1d(Z, W[b,0])));  q1 = relu(IN(conv1d(Z, W[b,1]) @ L))
                       X_b = q0 + q1                                  [2048, 48]
    A  = exp(Q @ K^T)                                                 [2048, 2048]
    P  = A / rowsum(A);  Aa = (P + P^T)/2;  out = Aa @ V              [2048, 48]

Strategy (8 NeuronCores, tensor-parallel over nhid):
    Core c owns output channels [c*256, (c+1)*256).  W is quantized on the
    host to fp8 e4m3 (x2^20 scale; instance norm makes the scale exact by
    using EPS*scale^2) with a data-aware error-feedback pass: weights are
    rounded sequentially per output channel, each to the fp8 neighbor that
    cancels the accumulated conv-output error against the actual stationary
    operand values.  The stationaries (padded-Z windows and ZcolL = Zcol@L)
    are stored as TWO-LEVEL fp8 (value = hi + lo, lo = fp8 of the residual)
    so the conv runs entirely in fp8 with DoubleRow perf mode (2 k-tiles
    per instruction, 0.5 cycles/row): bf16-level accuracy at fp8 bytes and
    2x PE rate.  W is read from HBM exactly once (28.3 MB/core).

    Schedule: three k-streaming sweeps K -> Q -> V.  K's all-gather hides
    under the Q sweep; kT / exp(Q K^T) / rowsums / exp(K Q^T) hide under
    the V sweep.  Tail: V epilogue, U = A^T (0.5 rinv V_loc) -> bf16
    ReduceScatter concurrent with the bf16 V all-gather ->
    0.5 rinv*(At^T-contract V_full) + U_scattered, store.
"""

import os
import sys

import numpy as np

sys.path.insert(0, "/opt/trn_rl_repo")

import orjson

import concourse.bass as bass
import concourse.mybir as mybir
from concourse import masks, tile
from concourse.bass_utils import run_bass_kernel_spmd

# ---------------------------------------------------------------- waitfix ---
# This neuronxcc build allows only ONE sync wait per instruction;
# TileContext emits instructions with several.  Rewrite the serialized BIR:
# hoist extra waits onto standalone NoOps inserted just before the
# instruction on the same engine (cumulative thresholds -> semantics kept).

_DMA_OPCODES = {
    "DMACopy", "DMATranspose", "TensorLoad", "TensorSave",
    "TriggeredCopy", "CollectiveCompute",
}
_wfix_counter = [0]


def _fix_block(instructions):
    out = []
    for ins in instructions:
        si = ins.get("sync_info")
        if not si:
            out.append(ins)
            continue
        waits = si.get("on_wait") or []
        updates = si.get("on_update") or []
        if len(waits) > 1:
            for w in waits[1:]:
                _wfix_counter[0] += 1
                out.append({
                    "engine": ins["engine"], "ins": [],
                    "name": f"WFIX-{_wfix_counter[0]}", "opcode": "NoOp",
                    "outs": [],
                    "sync_info": {"on_update": [], "on_wait": [w]},
                })
            si["on_wait"] = waits[:1]
        deferred = []
        if len(updates) > 1:
            assert ins.get("opcode", "") not in _DMA_OPCODES, (
                f"multi-update on DMA opcode: {ins['name']}"
            )
            si["on_update"] = updates[:1]
            for u in updates[1:]:
                _wfix_counter[0] += 1
                deferred.append({
                    "engine": ins["engine"], "ins": [],
                    "name": f"WFIX-{_wfix_counter[0]}", "opcode": "NoOp",
                    "outs": [],
                    "sync_info": {"on_update": [u], "on_wait": []},
                })
        out.append(ins)
        out.extend(deferred)
    return out


def _fix_bir_json_bytes(data: bytes) -> bytes:
    d = orjson.loads(data)
    for func in d.get("functions", []):
        for bb in func.get("blocks", []):
            bb["instructions"] = _fix_block(bb["instructions"])
    return orjson.dumps(d)


if not getattr(bass.Bass, "_waitfix_installed", False):
    _orig_to_json_bytes = bass.Bass.to_json_bytes

    def _patched_to_json_bytes(self) -> bytes:
        return _fix_bir_json_bytes(_orig_to_json_bytes(self))

    bass.Bass.to_json_bytes = _patched_to_json_bytes
    bass.Bass._waitfix_installed = True

# Synthesize the missing ``antenv.axon_hooks`` module so that
# ``run_bass_kernel_spmd(trace=True)`` can drive NTFF profiling through the
# axon PJRT plugin (the boot-time registration degrades silently when the
# module is absent).  Harmless when tracing is never requested.
try:
    import types

    import antenv

    if not hasattr(antenv, "axon_hooks"):
        _hooks_mod = types.ModuleType("antenv.axon_hooks")
        _ntff_hook = [None]
        _hooks_mod.set_axon_ntff_profile_hook = lambda h: _ntff_hook.__setitem__(0, h)
        _hooks_mod.get_axon_ntff_profile_hook = lambda: _ntff_hook[0]
        sys.modules["antenv.axon_hooks"] = _hooks_mod
        antenv.axon_hooks = _hooks_mod
        from trn_agent_boot.trn_boot import _ntff_profile_via_ctypes

        _hooks_mod.set_axon_ntff_profile_hook(
            _ntff_profile_via_ctypes("/opt/axon/libaxon_pjrt.so"))

    import concourse.bass_utils as _bu

    _bu.upload_artifacts = lambda tmpdir: tmpdir  # no fish share in container
except Exception:  # pragma: no cover - profiling is best-effort
    pass



# ------------------------------------------------------------- constants ---

NHID = 2048
NOPEN = 2048
N = 48          # spatial length
KD = 9          # conv kernel width
PAD = 4
NP = N + 2 * PAD            # 56 padded spatial
EPS = 1e-5
CORES = 8
OLOC = NHID // CORES        # 256 output channels per core
KTOT = KD * NOPEN           # 18432 contraction length
NKT = KTOT // 128           # 144 k-tiles
ISUB = NOPEN // 128         # 16 i-subtiles
SCALE = float(2.0 ** 20)    # fp8 weight scale; exact via EPS_S below
EPS_S = EPS * SCALE * SCALE
CK = 16                     # k-tiles per W DMA chunk (1 MB fp8)
NCH = NKT // CK             # 9 chunks per sweep
SCOLS = 512                 # cols per k-tile per sweep: [g_r0 | g_r1]
F32 = mybir.dt.float32
F32R = mybir.dt.float32r
BF16 = mybir.dt.bfloat16
FP8 = mybir.dt.float8e4
DR = mybir.MatmulPerfMode.DoubleRow


def _build_nc():
    nc = bass.Bass()

    wt_d = {b: nc.declare_dram_parameter(
        f"wt{b}", [NCH, 128, CK * SCOLS], FP8, isOutput=False)
        for b in range(3)}
    # pair-packed conv stationaries: slice for k-tile pair pi = t*8+sp is
    # [win(t, 2sp) | win(t, 2sp+1)], 96 wide (DoubleRow lhsT must be packed)
    zh_d = nc.declare_dram_parameter("zh", [128, NKT * N], FP8,
                                     isOutput=False)
    zl_d = nc.declare_dram_parameter("zl", [128, NKT * N], FP8,
                                     isOutput=False)
    zpt_d = nc.declare_dram_parameter("zpt", [NP, NOPEN], BF16, isOutput=False)
    lpd_d = nc.declare_dram_parameter("lpd", [NP, KD * N], BF16, isOutput=False)
    out_d = nc.declare_dram_parameter("out", [OLOC, N], F32, isOutput=True)

    with tile.TileContext(nc) as tc:
        with (
            tc.tile_pool(name="pers", bufs=1) as pers,
            tc.tile_pool(name="wpk", bufs=5) as wpk,
            tc.tile_pool(name="wpq", bufs=5) as wpq,
            tc.tile_pool(name="wpv", bufs=5) as wpv,
            tc.tile_pool(name="stats", bufs=1) as stats,
            tc.tile_pool(name="pacc", bufs=2, space="PSUM") as pacc,
            tc.tile_pool(name="ptrans", bufs=2, space="PSUM") as ptrans,
            tc.tile_pool(name="dram", bufs=1, space="DRAM") as dram,
        ):
            # ---------------- prologue ------------------------------------
            ident = pers.tile([128, 128], F32, tag="ident")
            masks.make_identity(nc, ident[:])
            identb = pers.tile([128, 128], BF16, tag="identb")
            nc.vector.tensor_copy(identb[:], ident[:])

            # rendezvous: absorb cross-core launch skew under the K sweep
            rg = [list(range(CORES))]
            rdv_in = dram.tile([2, 4], F32, tag="rdv_in")
            rdv_out = dram.tile([16, 4], F32, tag="rdv_out", addr_space="Shared")
            nc.gpsimd.collective_compute(
                "AllGather", mybir.AluOpType.bypass,
                replica_groups=rg, ins=[rdv_in.opt()], outs=[rdv_out.opt()])

            # two-level fp8 padded-Z windows (host-prepadded, pair-packed).
            # Issued on the sync queue AHEAD of the W chunk stream: on a
            # side queue they starve behind 6 MB of W prefetch and stall
            # the first conv matmul for ~18 us.
            zpt = pers.tile([128, NOPEN], BF16, tag="zpt")
            nc.sync.dma_start(out=zpt[0:NP, :], in_=zpt_d[:])
            lpd = pers.tile([128, KD * N], BF16, tag="lpd")
            nc.sync.dma_start(out=lpd[0:NP, :], in_=lpd_d[:])
            zh = pers.tile([128, NKT * N], FP8, tag="zh")
            nc.sync.dma_start(out=zh[:], in_=zh_d[:])
            zl = pers.tile([128, NKT * N], FP8, tag="zl")
            nc.sync.dma_start(out=zl[:], in_=zl_d[:])

            # ZcolL[(t,i), n'] = sum_n Zpad[i, n+t] L[n, n'], two-level fp8,
            # written pair-packed: [p][t, sp, parity, n]
            zch = pers.tile([128, NKT * N], FP8, tag="zch")
            zcl = pers.tile([128, NKT * N], FP8, tag="zcl")
            zch_w = zch[:].rearrange("p (t q r x) -> p t q r x",
                                     q=8, r=2, x=N)
            zcl_w = zcl[:].rearrange("p (t q r x) -> p t q r x",
                                     q=8, r=2, x=N)
            scr2 = [pers.tile([128, KD * N], F32, tag=f"zscr{i}",
                              name=f"zscr{i}") for i in range(2)]
            for s in range(ISUB):
                ps = ptrans.tile([128, KD * N], F32, tag="pzcl",
                                 name=f"pzcl{s}", bufs=2)
                nc.tensor.matmul(
                    ps[:, 0:KD * N],
                    zpt[0:NP, s * 128:(s + 1) * 128],
                    lpd[0:NP, :],
                    start=True, stop=True,
                )
                sp, par = s // 2, s % 2
                ps_v = ps[:, 0:KD * N].rearrange("p (t x) -> p t x", x=N)
                hi_ap = zch_w[:, :, sp, par, :]
                nc.scalar.copy(hi_ap, ps_v)
                scr_v = scr2[s % 2][:].rearrange("p (t x) -> p t x", x=N)
                nc.vector.tensor_tensor(scr_v, ps_v, hi_ap,
                                        op=mybir.AluOpType.subtract)
                nc.scalar.copy(zcl_w[:, :, sp, par, :], scr_v)

            zh_v = zh[:].rearrange("p (q x) -> p q x", x=N)    # [128,288,48]
            zl_v = zl[:].rearrange("p (q x) -> p q x", x=N)
            zch_v = zch[:].rearrange("p (q x) -> p q x", x=N)
            zcl_v = zcl[:].rearrange("p (q x) -> p q x", x=N)

            # ---------------- conv sweeps ---------------------------------
            relu_sc = pers.tile([128, 12 * N], F32, tag="relu_sc")
            yt_sb = pers.tile([128, 2 * 256], F32, tag="yt_sb")

            def conv_sweep(b, wpool_, accs, c0, c1, two_level):
                """One tensor's k-stream.  Per k-tile pair: one DoubleRow
                matmul per (branch, level); V uses the hi level only."""
                levels0 = (zh_v, zl_v) if two_level else (zh_v,)
                levels1 = (zch_v, zcl_v) if two_level else (zch_v,)
                nlev = len(levels0)
                for gch in range(c0, c1):
                    wt = wpool_.tile([128, CK * SCOLS], FP8, tag=f"w{b}",
                                     name=f"w{b}c{gch}")
                    nc.sync.dma_start(out=wt[:], in_=wt_d[b][gch])
                    wvv = wt[:].rearrange("p (j x) -> p j x", x=SCOLS)
                    for j in range(0, CK, 2):
                        kt = gch * CK + j
                        t, s = kt // ISUB, kt % ISUB
                        pi = t * 8 + s // 2
                        st = (kt == 0)
                        sp = (kt == NKT - 2)
                        for lv in range(nlev):
                            nc.tensor.matmul(
                                accs[0][0:N, :],
                                levels0[lv][:, 2 * pi:2 * pi + 2, :],
                                wvv[:, j:j + 2, 0:256],
                                start=(st and lv == 0),
                                stop=(sp and lv == nlev - 1),
                                perf_mode=DR)
                            nc.tensor.matmul(
                                accs[1][0:N, :],
                                levels1[lv][:, 2 * pi:2 * pi + 2, :],
                                wvv[:, j:j + 2, 256:512],
                                start=(st and lv == 0),
                                stop=(sp and lv == nlev - 1),
                                perf_mode=DR)

            def sweep_epilogue(entries, label):
                """entries: list of (g, acc_ap[48, 256]).  Transpose each
                half to [128, 48], then batched instance-norm stats + fused
                relu.  relu_sc slot = g*2 + h."""
                nslot = 2 * len(entries)
                xc = stats.tile([128, nslot * N], F32, tag=f"xc{label}",
                                name=f"xc{label}")
                slots = []
                for idx, (g, acc_ap) in enumerate(entries):
                    nc.scalar.copy(
                        yt_sb[0:N, idx * OLOC:(idx + 1) * OLOC], acc_ap)
                    for h in range(2):
                        ot = g * 2 + h
                        slot = idx * 2 + h
                        ps2 = ptrans.tile([128, 128], F32, tag="ptrans",
                                          name=f"tp{label}{ot}")
                        nc.tensor.transpose(
                            ps2[:, 0:N],
                            yt_sb[0:N, idx * OLOC + h * 128:
                                  idx * OLOC + (h + 1) * 128],
                            ident[0:N, 0:N])
                        nc.scalar.copy(xc[:, slot * N:(slot + 1) * N],
                                       ps2[:, 0:N])
                        slots.append((ot, slot))
                sm = stats.tile([128, nslot], F32, tag=f"sm{label}",
                                name=f"sm{label}")
                sq = stats.tile([128, nslot], F32, tag=f"sq{label}",
                                name=f"sq{label}")
                scr = stats.tile([128, nslot * N], F32, tag=f"scr{label}",
                                 name=f"scr{label}")
                for ot, slot in slots:
                    nc.vector.reduce_sum(
                        sm[:, slot:slot + 1], xc[:, slot * N:(slot + 1) * N],
                        axis=mybir.AxisListType.X)
                nc.vector.tensor_tensor(scr[:], xc[:], xc[:],
                                        op=mybir.AluOpType.mult)
                for ot, slot in slots:
                    nc.vector.reduce_sum(
                        sq[:, slot:slot + 1], scr[:, slot * N:(slot + 1) * N],
                        axis=mybir.AxisListType.X)
                mean = stats.tile([128, nslot], F32, tag=f"mean{label}",
                                  name=f"mean{label}")
                var = stats.tile([128, nslot], F32, tag=f"var{label}",
                                 name=f"var{label}")
                std = stats.tile([128, nslot], F32, tag=f"std{label}",
                                 name=f"std{label}")
                rsv = stats.tile([128, nslot], F32, tag=f"rsv{label}",
                                 name=f"rsv{label}")
                nb = stats.tile([128, nslot], F32, tag=f"nb{label}",
                                name=f"nb{label}")
                nc.vector.tensor_scalar_mul(mean[:], sm[:], 1.0 / N)
                nc.vector.tensor_scalar_mul(sq[:], sq[:], 1.0 / N)
                nc.vector.tensor_tensor(var[:], mean[:], mean[:],
                                        op=mybir.AluOpType.mult)
                nc.vector.tensor_tensor(var[:], sq[:], var[:],
                                        op=mybir.AluOpType.subtract)
                nc.vector.tensor_scalar_add(var[:], var[:], EPS_S)
                nc.scalar.sqrt(std[:], var[:])
                nc.vector.reciprocal(rsv[:], std[:])
                nc.vector.tensor_tensor(nb[:], mean[:], rsv[:],
                                        op=mybir.AluOpType.mult)
                nc.vector.tensor_scalar_mul(nb[:], nb[:], -1.0)
                for ot, slot in slots:
                    nc.scalar.activation(
                        relu_sc[:, ot * N:(ot + 1) * N],
                        xc[:, slot * N:(slot + 1) * N],
                        mybir.ActivationFunctionType.Relu,
                        bias=nb[:, slot:slot + 1], scale=rsv[:, slot:slot + 1])

            def branch_add(b, dst):
                for h in range(2):
                    ot0 = (2 * b) * 2 + h        # r = 0
                    ot1 = (2 * b + 1) * 2 + h    # r = 1
                    nc.vector.tensor_tensor(
                        dst[:, h * N:(h + 1) * N],
                        relu_sc[:, ot0 * N:(ot0 + 1) * N],
                        relu_sc[:, ot1 * N:(ot1 + 1) * N],
                        op=mybir.AluOpType.add)

            # ---- sweep 1: K (b=1); its gather hides under Q + V ----------
            accK = [pacc.tile([128, 256], F32, tag="acc", name=f"accK{i}")
                    for i in range(2)]
            conv_sweep(1, wpk, accK, 0, NCH, True)
            sweep_epilogue([(2, accK[0][0:N, :]), (3, accK[1][0:N, :])], "K")
            k_sb = pers.tile([128, 2 * N], BF16, tag="k_sb")
            branch_add(1, k_sb)

            kb = dram.tile([OLOC, N], BF16, tag="kb")
            kg = dram.tile([NHID, N], BF16, tag="kg", addr_space="Shared")
            nc.scalar.dma_start(
                out=kb[:].rearrange("(a p) n -> p a n", p=128),
                in_=k_sb[:].rearrange("p (a n) -> p a n", n=N))
            nc.gpsimd.collective_compute(
                "AllGather", mybir.AluOpType.bypass,
                replica_groups=rg, ins=[kb.opt()], outs=[kg.opt()])

            # ---- sweep 2: Q (b=0) ----------------------------------------
            accQ = [pacc.tile([128, 256], F32, tag="acc", name=f"accQ{i}")
                    for i in range(2)]
            conv_sweep(0, wpq, accQ, 0, NCH, True)
            sweep_epilogue([(0, accQ[0][0:N, :]), (1, accQ[1][0:N, :])], "Q")
            q_sb = pers.tile([128, 2 * N], BF16, tag="q_sb")
            branch_add(0, q_sb)

            qT = pers.tile([128, 2 * 128], BF16, tag="qT")
            for h in range(2):
                ps = ptrans.tile([128, 128], BF16, tag="ptrans", name=f"qT{h}")
                nc.tensor.transpose(
                    ps[0:N, :], q_sb[:, h * N:(h + 1) * N], identb[:])
                nc.scalar.copy(qT[0:N, h * 128:(h + 1) * 128], ps[0:N, :])

            kfull = pers.tile([128, 16 * N], BF16, tag="kfull")
            nc.gpsimd.dma_start(
                out=kfull[:].rearrange("p (a n) -> p a n", n=N),
                in_=kg[:].rearrange("(a p) n -> p a n", p=128))

            # ---- sweep 3: V (b=2, single-level) with attention prework ---
            accV = [pacc.tile([128, 256], F32, tag="acc", name=f"accV{i}")
                    for i in range(2)]
            conv_sweep(2, wpv, accV, 0, 5, False)

            # kT transposes: K gather is done by now
            kT = pers.tile([128, NHID], BF16, tag="kT")
            for jt in range(16):
                ps = ptrans.tile([128, 128], BF16, tag="ptrans",
                                 name=f"kT{jt}")
                nc.tensor.transpose(
                    ps[0:N, :], kfull[:, jt * N:(jt + 1) * N], identb[:])
                nc.scalar.copy(kT[0:N, jt * 128:(jt + 1) * 128], ps[0:N, :])

            conv_sweep(2, wpv, accV, 5, 6, False)

            # A rows: a_sb[m] = exp(Q_m K_full^T) (bf16) + rowsums
            a_sb = [pers.tile([128, NHID], BF16, tag=f"a{m}", name=f"a{m}")
                    for m in range(2)]
            rinvh = []
            for m in range(2):
                rspart = stats.tile([128, 4], F32, tag=f"rsp{m}",
                                    name=f"rsp{m}")
                for jc in range(4):
                    ps = ptrans.tile([128, 512], F32, tag="ptrans",
                                     name=f"am{m}c{jc}")
                    nc.tensor.matmul(
                        ps[:, 0:512],
                        qT[0:N, m * 128:(m + 1) * 128],
                        kT[0:N, jc * 512:(jc + 1) * 512],
                        start=True, stop=True)
                    nc.scalar.activation(
                        a_sb[m][:, jc * 512:(jc + 1) * 512], ps[:, 0:512],
                        mybir.ActivationFunctionType.Exp,
                        accum_out=rspart[:, jc:jc + 1])
                rowsum = stats.tile([128, 1], F32, tag=f"rowsum{m}",
                                    name=f"rowsum{m}")
                nc.vector.reduce_sum(rowsum[:], rspart[:],
                                     axis=mybir.AxisListType.X)
                rinv = stats.tile([128, 1], F32, tag=f"rinv{m}",
                                  name=f"rinv{m}")
                nc.vector.reciprocal(rinv[:], rowsum[:])
                rh = stats.tile([128, 1], F32, tag=f"rinvh{m}",
                                name=f"rinvh{m}")
                nc.vector.tensor_scalar_mul(rh[:], rinv[:], 0.5)
                rinvh.append(rh)

            conv_sweep(2, wpv, accV, 6, 7, False)

            # At = exp(K_full Q_loc^T) [2048, 256] (bf16)
            at_sb = pers.tile([128, 16 * 256], BF16, tag="at")
            for jt in range(16):
                ps = ptrans.tile([128, 256], F32, tag="ptrans",
                                 name=f"at{jt}")
                nc.tensor.matmul(
                    ps[:, 0:256],
                    kT[0:N, jt * 128:(jt + 1) * 128],
                    qT[0:N, 0:256],
                    start=True, stop=True)
                nc.scalar.activation(
                    at_sb[:, jt * 256:(jt + 1) * 256], ps[:, 0:256],
                    mybir.ActivationFunctionType.Exp)

            conv_sweep(2, wpv, accV, 7, NCH, False)

            # ---- V epilogue + tail ---------------------------------------
            sweep_epilogue([(4, accV[0][0:N, :]), (5, accV[1][0:N, :])], "V")
            vloc = pers.tile([128, 2 * N], BF16, tag="vloc")
            branch_add(2, vloc)

            # V all-gather (bf16), concurrent with U/ReduceScatter
            vb = dram.tile([OLOC, N], BF16, tag="vb")
            vg = dram.tile([NHID, N], BF16, tag="vg", addr_space="Shared")
            nc.scalar.dma_start(
                out=vb[:].rearrange("(a p) n -> p a n", p=128),
                in_=vloc[:].rearrange("p (a n) -> p a n", n=N))
            nc.gpsimd.collective_compute(
                "AllGather", mybir.AluOpType.bypass,
                replica_groups=rg, ins=[vb.opt()], outs=[vg.opt()])

            # U = A_loc^T (0.5 rinv V_loc);  ReduceScatter in bf16
            vr = pers.tile([128, 2 * N], BF16, tag="vr")
            for m in range(2):
                nc.vector.tensor_scalar_mul(
                    vr[:, m * N:(m + 1) * N], vloc[:, m * N:(m + 1) * N],
                    rinvh[m][:])
            u_sb = pers.tile([128, 16 * N], BF16, tag="u")
            for jt in range(16):
                ps = ptrans.tile([128, 128], F32, tag="ptrans",
                                 name=f"u{jt}")
                for m in range(2):
                    nc.tensor.matmul(
                        ps[:, 0:N],
                        a_sb[m][:, jt * 128:(jt + 1) * 128],
                        vr[:, m * N:(m + 1) * N],
                        start=(m == 0), stop=(m == 1))
                u_copy = (nc.vector.tensor_copy if jt % 2 == 0
                          else nc.scalar.copy)
                u_copy(u_sb[:, jt * N:(jt + 1) * N], ps[:, 0:N])

            ub = dram.tile([NHID, N], BF16, tag="ub")
            rsb = dram.tile([OLOC, N], BF16, tag="rsb")
            nc.scalar.dma_start(
                out=ub[:].rearrange("(a p) n -> p a n", p=128),
                in_=u_sb[:].rearrange("p (a n) -> p a n", n=N))
            nc.gpsimd.collective_compute(
                "ReduceScatter", mybir.AluOpType.add,
                replica_groups=rg, ins=[ub.opt()], outs=[rsb.opt()])

            vfull = pers.tile([128, 16 * N], BF16, tag="vfull")
            nc.gpsimd.dma_start(
                out=vfull[:].rearrange("p (a n) -> p a n", n=N),
                in_=vg[:].rearrange("(a p) n -> p a n", p=128))

            rs_bf = pers.tile([128, 2 * N], BF16, tag="rs_bf")
            nc.gpsimd.dma_start(
                out=rs_bf[:].rearrange("p (a n) -> p a n", n=N),
                in_=rsb[:].rearrange("(a p) n -> p a n", p=128))
            rs_sb = pers.tile([128, 2 * N], F32, tag="rs_sb")
            nc.vector.tensor_copy(rs_sb[:], rs_bf[:])

            # out = 0.5 rinv * (At^T-contract V_full) + rs
            fin = pers.tile([128, 2 * N], F32, tag="fin")
            for m in range(2):
                ps = ptrans.tile([128, 128], F32, tag="ptrans",
                                 name=f"fin{m}")
                for jt in range(16):
                    nc.tensor.matmul(
                        ps[:, 0:N],
                        at_sb[:, jt * 256 + m * 128: jt * 256 + (m + 1) * 128],
                        vfull[:, jt * N:(jt + 1) * N],
                        start=(jt == 0), stop=(jt == 15))
                nc.vector.scalar_tensor_tensor(
                    out=fin[:, m * N:(m + 1) * N],
                    in0=ps[:, 0:N],
                    scalar=rinvh[m][:],
                    in1=rs_sb[:, m * N:(m + 1) * N],
                    op0=mybir.AluOpType.mult,
                    op1=mybir.AluOpType.add)

            nc.scalar.dma_start(
                out=out_d[:].rearrange("(a p) n -> p a n", p=128),
                in_=fin[:].rearrange("p (a n) -> p a n", n=N))

    return nc


_NC_CACHE = None


def _get_nc():
    global _NC_CACHE
    if _NC_CACHE is None:
        _NC_CACHE = _build_nc()
    return _NC_CACHE


# ------------------------------------------------------- host-side prep ---

def _fp8_cast(x):
    import ml_dtypes

    return np.asarray(x, dtype=ml_dtypes.float8_e4m3)


def _fp8_lo_hi(x):
    """fp8(e4m3) grid values bracketing x (f32 in, f32 out, exact grid)."""
    import ml_dtypes

    dt8 = ml_dtypes.float8_e4m3
    q8 = np.asarray(x, dtype=dt8)
    q = q8.astype(np.float32)
    b = q8.view(np.uint8)
    neg = (b & 0x80) != 0
    up_b = np.where(~neg, b + 1,
                    np.where(b == 0x80, np.uint8(1), b - 1)).astype(np.uint8)
    dn_b = np.where(neg, b + 1,
                    np.where(b == 0, np.uint8(0x81), b - 1)).astype(np.uint8)
    up = up_b.view(dt8).astype(np.float32)
    dn = dn_b.view(dt8).astype(np.float32)
    lo = np.where(q <= x, q, dn)
    hi = np.where(q >= x, q, up)
    return lo, hi


def _make_feedback_numba():
    import numba

    @numba.njit(fastmath=True, boundscheck=False, cache=False)
    def _feedback(Ws, lo, hi, stat, out):
        K, C = Ws.shape
        Nn = stat.shape[1]
        E = np.zeros((C, Nn), np.float32)
        for k in range(K):
            zk = stat[k]
            zz = np.float32(0.0)
            for n in range(Nn):
                zz += zk[n] * zk[n]
            for c in range(C):
                w = Ws[k, c]
                dlo = lo[k, c] - w
                dhi = hi[k, c] - w
                ez = np.float32(0.0)
                Ec = E[c]
                for n in range(Nn):
                    ez += Ec[n] * zk[n]
                if 2 * dlo * ez + dlo * dlo * zz <= 2 * dhi * ez + dhi * dhi * zz:
                    d = dlo
                    out[k, c] = lo[k, c]
                else:
                    d = dhi
                    out[k, c] = hi[k, c]
                for n in range(Nn):
                    Ec[n] += d * zk[n]
        return out

    return _feedback


def _feedback_numpy(Ws, lo, hi, stat, out, nseg=9):
    """Vectorized fallback: segment the k axis, run feedback per segment in
    parallel (slightly worse residual than fully sequential)."""
    K, C = Ws.shape
    Nn = stat.shape[1]
    T = K // nseg
    Wseg = Ws.reshape(nseg, T, C)
    Sseg = stat.reshape(nseg, T, Nn)
    Lseg = lo.reshape(nseg, T, C)
    Hseg = hi.reshape(nseg, T, C)
    Oseg = out.reshape(nseg, T, C)
    E = np.zeros((nseg, C, Nn), np.float32)
    for k in range(T):
        zk = Sseg[:, k]
        dlo = Lseg[:, k] - Wseg[:, k]
        dhi = Hseg[:, k] - Wseg[:, k]
        ez = np.einsum('scn,sn->sc', E, zk)
        zz = (zk * zk).sum(-1)[:, None]
        pick_lo = (2 * dlo * ez + dlo * dlo * zz) <= (2 * dhi * ez + dhi * dhi * zz)
        d = np.where(pick_lo, dlo, dhi)
        Oseg[:, k] = np.where(pick_lo, Lseg[:, k], Hseg[:, k])
        E += d[:, :, None] * zk[:, None, :]
    return out


def _prep_inputs(Z: np.ndarray, L: np.ndarray, W: np.ndarray):
    """Host-side layout + fp8 quantization with conv-output error feedback."""
    import ml_dtypes

    bf = ml_dtypes.bfloat16

    # padded Z, two-level fp8, pair-packed chip layout
    # zh[p, ((t*8+sp)*2+par)*48 + n] = hi(Zpad)[(2sp+par)*128 + p, t + n]
    Zpad = np.zeros((NOPEN, NP), np.float32)
    Zpad[:, PAD:PAD + N] = Z
    z_hi8 = _fp8_cast(Zpad)
    z_hi = z_hi8.astype(np.float32)
    z_lo8 = _fp8_cast(Zpad - z_hi)
    z2 = z_hi + z_lo8.astype(np.float32)        # stationary value on chip

    def pairpack(x8):
        zw = np.empty((128, KD, 8, 2, N), dtype=x8.dtype)
        for t in range(KD):
            for s in range(ISUB):
                zw[:, t, s // 2, s % 2, :] = x8[s * 128:(s + 1) * 128, t:t + N]
        return np.ascontiguousarray(zw.reshape(128, NKT * N))

    zh = pairpack(z_hi8)
    zl = pairpack(z_lo8)

    Zpad_bf = Zpad.astype(bf)
    zpt = np.ascontiguousarray(Zpad_bf.T)                  # [56, 2048]
    lpd = np.zeros((NP, KD * N), np.float32)
    for t in range(KD):
        lpd[t:t + N, t * N:(t + 1) * N] = L
    lpd_bf = lpd.astype(bf)

    # stationary rows as used on chip, k = t*NOPEN + i.  K/Q convs see the
    # two-level value; the V conv uses the hi level only.
    stat0_2 = np.empty((KTOT, N), np.float32)
    stat0_1 = np.empty((KTOT, N), np.float32)
    for t in range(KD):
        stat0_2[t * NOPEN:(t + 1) * NOPEN] = z2[:, t:t + N]
        stat0_1[t * NOPEN:(t + 1) * NOPEN] = z_hi[:, t:t + N]
    # ZcolL replicated: psum f32 of bf16 matmul -> two-level fp8
    zc = (zpt.astype(np.float32).T @ lpd_bf.astype(np.float32))  # [2048, 432]
    zc_hi8 = _fp8_cast(zc)
    zc_hi = zc_hi8.astype(np.float32)
    zc_lo8 = _fp8_cast(zc - zc_hi)
    zc2 = (zc_hi + zc_lo8.astype(np.float32)).reshape(NOPEN, KD, N)
    stat1_2 = np.ascontiguousarray(zc2.transpose(1, 0, 2).reshape(KTOT, N))
    stat1_1 = np.ascontiguousarray(
        zc_hi.reshape(NOPEN, KD, N).transpose(1, 0, 2).reshape(KTOT, N))

    # W -> [k, g, ch] scaled, feedback-quantize per group-set against the
    # stationary values that conv actually uses
    Wt = np.ascontiguousarray(
        W.transpose(4, 3, 0, 1, 2).reshape(KTOT, 6, NHID).astype(np.float32))
    Wt *= SCALE

    try:
        fb = _make_feedback_numba()
        use_numba = True
    except Exception:
        fb = None
        use_numba = False

    Wq = np.empty((KTOT, 6, NHID), np.float32)
    for gsel, stat in (([0, 2], stat0_2), ([1, 3], stat1_2),
                       ([4], stat0_1), ([5], stat1_1)):
        Ws = np.ascontiguousarray(
            Wt[:, gsel, :].reshape(KTOT, len(gsel) * NHID))
        lo, hi = _fp8_lo_hi(Ws)
        outq = np.empty_like(Ws)
        if use_numba:
            fb(Ws, lo, hi, stat, outq)
        else:
            _feedback_numpy(Ws, lo, hi, stat, outq)
        Wq[:, gsel, :] = outq.reshape(KTOT, len(gsel), NHID)
        del Ws, lo, hi, outq
    del Wt

    q8 = _fp8_cast(Wq)           # exact: values are on the fp8 grid
    del Wq

    shards = []
    for c in range(CORES):
        wc = q8[:, :, c * OLOC:(c + 1) * OLOC]       # [k, 6, 256]
        slabs = []
        for b in range(3):
            wb = np.ascontiguousarray(
                wc[:, [2 * b, 2 * b + 1], :]).reshape(NKT, 128, SCOLS)
            wb = np.ascontiguousarray(
                wb.reshape(NCH, CK, 128, SCOLS).transpose(0, 2, 1, 3)
                .reshape(NCH, 128, CK * SCOLS))
            slabs.append(wb)
        shards.append(slabs)

    return (shards, np.ascontiguousarray(zh), np.ascontiguousarray(zl),
            np.ascontiguousarray(zpt), np.ascontiguousarray(lpd_bf))


def kernel(Z: np.ndarray, L: np.ndarray, W: np.ndarray) -> np.ndarray:
    nc = _get_nc()
    shards, zh, zl, zpt, lpd = _prep_inputs(
        np.asarray(Z, dtype=np.float32), np.asarray(L, dtype=np.float32),
        np.asarray(W, dtype=np.float32))
    in_maps = [{"wt0": shards[c][0], "wt1": shards[c][1],
                "wt2": shards[c][2],
                "zh": zh, "zl": zl, "zpt": zpt, "lpd": lpd}
               for c in range(CORES)]
    trace = bool(int(os.environ.get("KERNEL_TRACE", "0")))
    kw = {}
    if trace and int(os.environ.get("KERNEL_TRACE_ALL", "0")):
        kw["trace_cores"] = list(range(CORES))
    res = run_bass_kernel_spmd(nc, in_maps, list(range(CORES)), trace=trace,
                               **kw)
    kernel.last_result = res
    out = np.concatenate([res.results[c]["out"] for c in range(CORES)], axis=0)
    return out
